# revision 12
# baseline (speedup 1.0000x reference)
"""KitNET (nn_KitNET_35287451304350) Trainium2 kernel, v2.

Data-parallel over batch across 8 NeuronCores; each core gets B/8 = 65536
rows. The host pre-gathers/normalizes x and ships it bf16 *feature-major*
([102, rows] per core), so the device pipeline has no transpose and no
PSUM->SBUF evacuation pass:

  per super-block of NB=1024 batch columns:
    PE  : He = W1bd.T @ xn            (block-diag enc, 102->85, 2x N=512)
    ACT : h  = sigmoid(He + b1)       (PSUM->SBUF, per-partition bias)
    PE  : Yp = W2bd.T @ h             (block-diag dec, 85->102)
    ACT : y  = sigmoid(Yp + b2)
    DVE : diff = y - xn               (tensor_tensor, bf16 2x mode)
    DVE : acc[:,i] = sum_free(diff*diff)   (tensor_tensor_reduce, one pass)
  tail: acc[102, nsuper] -> reduce -> partials[102] -> DRAM

The loop is software-pipelined (h of block i+1 is queued on ACT before y of
block i) so the ACT engine -- the throughput bound at 2 sigmoid passes per
batch row -- never waits on the PE.

Host combines the 8 partial sum vectors into per-cluster RMSE and runs the
tiny 17->13->17 head autoencoder in numpy (microseconds of work).
"""

import os
import sys

import numpy as np

sys.path.insert(0, "/opt/trn_rl_repo")

import concourse.bass as bass
import concourse.bacc as bacc
import concourse.mybir as mybir
from concourse.tile import TileContext
from concourse.bass_utils import run_bass_kernel_spmd

# problem constants (hardcoded per harness contract)
B, D, C, F, H = 524288, 102, 17, 6, 5
NCORES = 8
BS = B // NCORES          # rows per core
EPS = 1e-16

# tunables (env-overridable for A/B during development)
NB = int(os.environ.get("KITNET_NB", "1024"))          # batch cols per super-block
DMAC = int(os.environ.get("KITNET_DMAC", "8192"))      # batch cols per input DMA
MMN = int(os.environ.get("KITNET_MMN", "512"))         # matmul moving free dim


USE_TTR = os.environ.get("KITNET_TTR", "stt")  # "stt" | "plain" | "ttr"(broken on HW)


def build_nc(nb: int = NB, dmac: int = DMAC, rows: int = BS,
             use_ttr: str = USE_TTR) -> bass.Bass:
    f32 = mybir.dt.float32
    bf16 = mybir.dt.bfloat16
    nsuper = rows // nb
    sb_per_dma = dmac // nb
    nmm = nb // MMN

    nc = bacc.Bacc()
    xn_d = nc.declare_dram_parameter("xn", [D, rows], bf16, isOutput=False)
    w1_d = nc.declare_dram_parameter("w1", [D, C * H], bf16, isOutput=False)
    w2_d = nc.declare_dram_parameter("w2", [C * H, D], bf16, isOutput=False)
    cvec_d = nc.declare_dram_parameter("cvec", [D, 8], f32, isOutput=False)
    partials = nc.declare_dram_parameter("partials", [D, 1], f32, isOutput=True)

    SIG = mybir.ActivationFunctionType.Sigmoid
    SUB = mybir.AluOpType.subtract
    MUL = mybir.AluOpType.mult
    ADD = mybir.AluOpType.add

    with TileContext(nc) as tc:
        with (
            tc.tile_pool(name="consts", bufs=1) as cpool,
            tc.tile_pool(name="xin", bufs=2) as xpool,
            tc.tile_pool(name="hp", bufs=2) as hpool,
            tc.tile_pool(name="yp", bufs=2) as ypool,
            tc.tile_pool(name="dp", bufs=2) as dpool,
            tc.tile_pool(name="sq", bufs=2) as sqpool,
            tc.tile_pool(name="ps_h", bufs=2, space="PSUM") as psh,
            tc.tile_pool(name="ps_y", bufs=2, space="PSUM") as psy,
        ):
            w1_sb = cpool.tile([D, C * H], bf16)
            nc.sync.dma_start(out=w1_sb[:], in_=w1_d[:])
            w2_sb = cpool.tile([C * H, D], bf16)
            nc.sync.dma_start(out=w2_sb[:], in_=w2_d[:])
            cvec_sb = cpool.tile([D, 8], f32)
            nc.sync.dma_start(out=cvec_sb[:], in_=cvec_d[:])
            b2_sb = cvec_sb[:, 0:1]
            b1_sb = cvec_sb[: C * H, 1:2]

            acc = cpool.tile([D, nsuper], f32)

            # software-pipelined over superblocks: stage A (enc+sigmoid_h) of
            # block i is emitted before stage B (dec+sigmoid_y+mse) of i-1,
            # so each engine's FIFO always has ready work queued.
            xts = [None] * nsuper
            hs = [None] * nsuper
            ys = [None] * nsuper
            for i in range(nsuper + 1):
                if i < nsuper:
                    if i % sb_per_dma == 0:
                        xt = xpool.tile([D, dmac], bf16)
                        nc.sync.dma_start(
                            out=xt[:],
                            in_=xn_d[:, i * nb : i * nb + dmac],
                        )
                        for k in range(sb_per_dma):
                            xts[i + k] = xt[:, k * nb : (k + 1) * nb]
                    he = psh.tile([C * H, nb], f32)
                    for m in range(nmm):
                        sl = slice(m * MMN, (m + 1) * MMN)
                        nc.tensor.matmul(
                            he[:, sl], w1_sb[:], xts[i][:, sl],
                            start=True, stop=True,
                        )
                    h = hpool.tile([C * H, nb], bf16)
                    nc.scalar.activation(h[:], he[:], SIG, bias=b1_sb, scale=1.0)
                    hs[i] = h
                if i >= 1:
                    j = i - 1
                    yp = psy.tile([D, nb], f32)
                    for m in range(nmm):
                        sl = slice(m * MMN, (m + 1) * MMN)
                        nc.tensor.matmul(
                            yp[:, sl], w2_sb[:], hs[j][:, sl],
                            start=True, stop=True,
                        )
                    y = ypool.tile([D, nb], bf16)
                    nc.scalar.activation(y[:], yp[:], SIG, bias=b2_sb, scale=1.0)
                    ys[j] = y
                    diff = dpool.tile([D, nb], bf16)
                    nc.vector.tensor_tensor(diff[:], y[:], xts[j][:], SUB)
                    if use_ttr == "ttr":
                        d2 = sqpool.tile([D, nb], bf16)
                        nc.vector.tensor_tensor_reduce(
                            out=d2[:], in0=diff[:], in1=diff[:],
                            scale=1.0, scalar=0.0, op0=MUL, op1=ADD,
                            accum_out=acc[:, j : j + 1],
                        )
                    elif use_ttr == "stt":
                        d2 = sqpool.tile([D, nb], bf16)
                        nc.vector.scalar_tensor_tensor(
                            out=d2[:], in0=diff[:], scalar=1.0, in1=diff[:],
                            op0=MUL, op1=MUL,
                            accum_out=acc[:, j : j + 1],
                        )
                    else:
                        d2 = sqpool.tile([D, nb], bf16)
                        nc.vector.tensor_mul(d2[:], diff[:], diff[:])
                        nc.vector.reduce_sum(
                            out=acc[:, j : j + 1], in_=d2[:],
                            axis=mybir.AxisListType.X,
                        )
                    hs[j] = None
                    ys[j] = None

            accsum = cpool.tile([D, 1], f32)
            nc.vector.reduce_sum(out=accsum[:], in_=acc[:], axis=mybir.AxisListType.X)
            nc.sync.dma_start(out=partials[:], in_=accsum[:])

    nc.compile()
    return nc


_NC_CACHE: dict = {}


def _get_nc(nb=NB, dmac=DMAC):
    key = (nb, dmac)
    if key not in _NC_CACHE:
        _NC_CACHE[key] = build_nc(nb, dmac)
    return _NC_CACHE[key]


def _prep_in_maps(x, clusters_idx, norm_min, norm_max, enc_w, enc_b, dec_w, dec_b):
    import ml_dtypes

    x = np.asarray(x, dtype=np.float32)
    ci = np.asarray(clusters_idx).ravel()
    if not np.array_equal(ci, np.arange(D)):
        x = np.take(x, ci, axis=1)

    mn = np.asarray(norm_min, np.float32).ravel()
    rng = np.asarray(norm_max, np.float32).ravel() - mn + np.float32(EPS)
    sc = (np.float32(1.0) / rng).astype(np.float32)

    # normalize + cast row-major (contiguous, vectorized), then one
    # feature-major transpose copy per core shard.
    xn = ((x - mn[None, :]) * sc[None, :]).astype(ml_dtypes.bfloat16)

    enc_w = np.asarray(enc_w, np.float32)
    dec_w = np.asarray(dec_w, np.float32)
    W1 = np.zeros((D, C * H), np.float32)
    W2 = np.zeros((C * H, D), np.float32)
    for c in range(C):
        W1[c * F : (c + 1) * F, c * H : (c + 1) * H] = enc_w[c].T  # [F,H]
        W2[c * H : (c + 1) * H, c * F : (c + 1) * F] = dec_w[c].T  # [H,F]
    W1 = W1.astype(ml_dtypes.bfloat16)
    W2 = W2.astype(ml_dtypes.bfloat16)

    cvec = np.zeros((D, 8), np.float32)
    cvec[:, 0] = np.asarray(dec_b, np.float32).ravel()
    cvec[: C * H, 1] = np.asarray(enc_b, np.float32).ravel()

    const = dict(w1=W1, w2=W2, cvec=cvec)
    in_maps = []
    for i in range(NCORES):
        m = dict(const)
        m["xn"] = np.ascontiguousarray(xn[i * BS : (i + 1) * BS].T)
        in_maps.append(m)
    return in_maps


def run_device(in_maps, nb=NB, dmac=DMAC, trace=False, **kw):
    nc = _get_nc(nb, dmac)
    return run_bass_kernel_spmd(nc, in_maps, list(range(NCORES)), trace=trace, **kw)


def _finish_host(partials_per_core, head_enc_w, head_enc_b, head_dec_w, head_dec_b,
                 out_min, out_max):
    tot = np.zeros(D, np.float64)
    for p in partials_per_core:
        tot += np.asarray(p, np.float64).ravel()
    mse = tot.reshape(C, F).sum(axis=1) / (B * F)
    tails = np.sqrt(mse).astype(np.float32)
    tails = np.where(tails == 0.0, np.float32(0.01), tails).astype(np.float32)
    om = np.float32(np.asarray(out_min).ravel()[0])
    ox = np.float32(np.asarray(out_max).ravel()[0])
    tails = ((tails - om) / (ox - om + np.float32(EPS))).astype(np.float32)

    hew = np.asarray(head_enc_w, np.float32)
    heb = np.asarray(head_enc_b, np.float32)
    hdw = np.asarray(head_dec_w, np.float32)
    hdb = np.asarray(head_dec_b, np.float32)

    def sig(v):
        return (1.0 / (1.0 + np.exp(-v.astype(np.float32)))).astype(np.float32)

    hh = sig(hew @ tails + heb)
    out = sig(hdw @ hh + hdb)
    return out.astype(np.float32), tails.astype(np.float32)


def kernel(x, clusters_idx, norm_min, norm_max, enc_w, enc_b, dec_w, dec_b,
           head_enc_w, head_enc_b, head_dec_w, head_dec_b, out_min, out_max):
    in_maps = _prep_in_maps(
        x, clusters_idx, norm_min, norm_max, enc_w, enc_b, dec_w, dec_b
    )
    res = run_device(in_maps)
    partials = [res.results[i]["partials"] for i in range(NCORES)]
    return _finish_host(
        partials, head_enc_w, head_enc_b, head_dec_w, head_dec_b, out_min, out_max
    )
